# revision 51
# baseline (speedup 1.0000x reference)
"""Trainium2 Bass kernel for nn_MiniDSARouter (topk block routing).

Problem (hardcoded shapes): B=2, T=8192, HQ=32, H=8, D=64, DR=16,
block_size=64, selected_blocks=16, groups=4, ADD_LOCAL=1.

Reference semantics (verified equivalent):
  out[b,t,h,:] = sorted(top16_idx(scores[b,t,h,:]) ++ [t_blk, max(t_blk-1,0)])[:16]
where scores = (Qrep @ Wq) @ (blockmean(K) @ Wk)^T with causal block mask
(blocks > t//64 masked to -inf), and the positive per-head scale
exp(logit_scale) never changes the ranking so it is dropped.
The sequential "dedup" in the reference is numerically a no-op.
For t_blk <= 15 the top-16 set is always {0..15} (mask fill), so those rows
are a static function of t and are written from a precomputed table.

Sharding: one KV head per NeuronCore (8 heads / 8 cores). All work is
per-(b,t,h) so there is no cross-core communication.
"""

import numpy as np

import concourse.bass as bass
import concourse.mybir as mybir
import concourse.tile as tile
from concourse import bacc
from concourse.bass_utils import run_bass_kernel_spmd

B, T, HQ, H, D, DR = 2, 8192, 32, 8, 64, 16
BS = 64                    # block size
NB = T // BS               # 128 blocks per batch
SEL = 16                   # selected blocks
GROUPS = 4
ROWS = B * T               # 16384 rows per core (all t of both batches, one head)
NTILES_SKIP = 8            # per-batch tiles 0..7 (t < 1024) are static
TPB = T // 128             # 64 row-tiles of 128 per batch
NGT = TPB - NTILES_SKIP    # 56 computed tiles per batch
NG = B * NGT               # 112 computed tiles per core
NEG_BIG = -1e30

_CACHE = {}


def _tiles():
    """Computed tiles: g -> (b, i). Tile covers rows b*T + i*128 ... +128."""
    return [(g // NGT, NTILES_SKIP + g % NGT) for g in range(NG)]


def _static_tables():
    # early rows: t_blk <= 15 -> sorted([0..15] + [t_blk, max(t_blk-1,0)])[:16]
    # laid out [p, j, :] = row t = j*128 + p  (SBUF partition tiling)
    early = np.empty((128, NTILES_SKIP, SEL), np.int32)
    for t in range(NTILES_SKIP * 128):
        tb = t // BS
        s = sorted(list(range(16)) + [tb, max(tb - 1, 0)])
        early[t % 128, t // 128] = s[:SEL]
    early = early.reshape(128, NTILES_SKIP * SEL)
    # loc1[p, g] = t_blk - 1 for the row at (partition p, computed tile g):
    # final clamp out[15] = min(out[15], t_blk - 1)
    loc1 = np.empty((128, NG), np.uint32)
    for g, (b, i) in enumerate(_tiles()):
        for p in range(128):
            loc1[p, g] = 2 * i + (1 if p >= 64 else 0) - 1
    # descending constants for max_index on the prefix-scan: [-1..-16]
    cdesc = np.tile(-np.arange(1, 17, dtype=np.float32), (128, 1))
    # zap column: add -1e30 to (p < 64) rows of the tile's odd column
    zap = np.zeros((1, 128), np.float32)
    zap[0, :64] = NEG_BIG
    one = np.ones((1, 1), np.float32)
    return early, loc1, cdesc, zap, one


def build_program():
    f32 = mybir.dt.float32
    nc = bacc.Bacc("TRN2", target_bir_lowering=False, debug=False)

    qT_d = nc.dram_tensor("qT", [D, ROWS], f32, kind="ExternalInput")
    kT_d = nc.dram_tensor("kT", [D, ROWS], f32, kind="ExternalInput")
    wq_d = nc.dram_tensor("wq", [D, DR], f32, kind="ExternalInput")
    wks_d = nc.dram_tensor("wks", [D, DR], f32, kind="ExternalInput")
    loc1_d = nc.dram_tensor("loc1", [128, NG], mybir.dt.uint32, kind="ExternalInput")
    cdesc_d = nc.dram_tensor("cdesc", [128, SEL], f32, kind="ExternalInput")
    zap_d = nc.dram_tensor("zap", [1, 128], f32, kind="ExternalInput")
    one_d = nc.dram_tensor("one", [1, 1], f32, kind="ExternalInput")
    early_d = nc.dram_tensor("early", [128, NTILES_SKIP * SEL], mybir.dt.uint32,
                             kind="ExternalInput")
    out_d = nc.dram_tensor("out", [ROWS, SEL], mybir.dt.uint32,
                           kind="ExternalOutput")

    with tile.TileContext(nc) as tc:
        with (
            tc.tile_pool(name="singles", bufs=1) as singles,
            tc.tile_pool(name="kchunk", bufs=2) as kpool,
            tc.tile_pool(name="qchunk", bufs=6) as qpool,
            tc.tile_pool(name="tree", bufs=1) as tree,
            tc.tile_pool(name="qr_ps", bufs=2, space="PSUM") as qr_ps,
            tc.tile_pool(name="sc_ps", bufs=6, space="PSUM") as sc_ps,
            tc.tile_pool(name="sc_sb", bufs=16) as scpool,
            tc.tile_pool(name="small", bufs=12) as small,
            tc.tile_pool(name="ogrp", bufs=2) as ogpool,
        ):
            GB = 14
            # ---- static tables / params ----
            wq_sb = singles.tile([D, DR], f32)
            nc.sync.dma_start(out=wq_sb, in_=wq_d.ap())
            wks_sb = singles.tile([D, DR], f32)
            nc.sync.dma_start(out=wks_sb, in_=wks_d.ap())
            zap_sb = singles.tile([1, 128], f32)
            nc.sync.dma_start(out=zap_sb, in_=zap_d.ap())
            one_sb = singles.tile([1, 1], f32)
            nc.sync.dma_start(out=one_sb, in_=one_d.ap())
            out_v = out_d.ap().rearrange("(j p) s -> p j s", p=128)

            # ---- block means: ksumT[d, n] = sum_t K[t in block n, d] ----
            # (the 1/64 of the mean is folded into wks on the host)
            ksumT = singles.tile([D, B * NB], f32)
            krT_sb = singles.tile([DR, B * NB], f32)
            CH = 4096                      # 64 blocks per chunk
            BLK_CH = CH // BS

            def ksum_steps(c, eng):
                """DMA the chunk now; return per-level thunks + kr finisher."""
                kc = kpool.tile([D, BLK_CH, BS], f32)
                half = CH // 2
                nc.sync.dma_start(out=kc[:, :BLK_CH // 2, :],
                                  in_=kT_d.ap()[:, c * CH:c * CH + half])
                nc.sync.dma_start(out=kc[:, BLK_CH // 2:, :],
                                  in_=kT_d.ap()[:, c * CH + half:(c + 1) * CH])
                steps = []
                state = {"cur": kc, "w": BS}

                def level_part(nsub, sub):
                    def f():
                        w = state["w"] // 2
                        cur = state["cur"]
                        blo = BLK_CH * sub // nsub
                        bhi = BLK_CH * (sub + 1) // nsub
                        if w == 1:
                            dst = ksumT[:, c * BLK_CH + blo:c * BLK_CH + bhi]
                            eng.tensor_add(dst, cur[:, blo:bhi, 0:1],
                                           cur[:, blo:bhi, 1:2])
                        else:
                            if sub == 0:
                                nxt_tile = tree.tile([D, BLK_CH, w], f32,
                                                     name=f"tr{c}_{w}",
                                                     tag=f"tree{c%2}_{w}")
                                state["nxt"] = nxt_tile
                            nxt = state["nxt"]
                            eng.tensor_add(nxt[:, blo:bhi, :],
                                           cur[:, blo:bhi, 0:w],
                                           cur[:, blo:bhi, w:2 * w])
                        if sub == nsub - 1:
                            if w > 1:
                                state["cur"] = state["nxt"]
                            state["w"] = w
                    return f

                def finish():
                    kr_psum = sc_ps.tile([DR, BLK_CH], f32, tag="scps")
                    nc.tensor.matmul(kr_psum, lhsT=wks_sb,
                                     rhs=ksumT[:, c * BLK_CH:(c + 1) * BLK_CH],
                                     start=True, stop=True)
                    nc.scalar.copy(out=krT_sb[:, c * BLK_CH:(c + 1) * BLK_CH],
                                   in_=kr_psum)
                for w, nsub in ((32, 4), (16, 2), (8, 1), (4, 1), (2, 1), (1, 1)):
                    for sub in range(nsub):
                        steps.append(level_part(nsub, sub))
                steps.append(finish)
                return steps

            def ksum_chunk(c, eng):
                for s in ksum_steps(c, eng):
                    s()

            ksum_chunk(0, nc.vector)

            # non-critical tables + static early rows (queued after the
            # critical kT chunk 0 / qT loads)
            loc1_sb = singles.tile([128, NG], mybir.dt.uint32)
            nc.sync.dma_start(out=loc1_sb, in_=loc1_d.ap())
            cdesc_sb = singles.tile([128, SEL], f32)
            nc.sync.dma_start(out=cdesc_sb, in_=cdesc_d.ap())
            early_sb = singles.tile([128, NTILES_SKIP, SEL], mybir.dt.uint32)
            nc.sync.dma_start(out=early_sb, in_=early_d.ap())
            for b in range(B):
                jb = b * TPB
                nc.sync.dma_start(out=out_v[:, jb:jb + NTILES_SKIP, :],
                                  in_=early_sb)
            # ---- qrT[r, row] = Wq^T @ qT, computed lazily per 512-col chunk
            # (interleaved with the score tiles; chunks covering only
            #  never-scored early rows are skipped entirely)
            qrT_sb = singles.tile([DR, ROWS], f32)
            QC = 512
            qr_done = set()

            def ensure_qr(col_lo, col_hi):
                for c in range(col_lo // QC, (col_hi + QC - 1) // QC):
                    if c in qr_done:
                        continue
                    qr_done.add(c)
                    qc = qpool.tile([D, QC], f32)
                    nc.sync.dma_start(out=qc,
                                      in_=qT_d.ap()[:, c * QC:(c + 1) * QC])
                    ps = qr_ps.tile([DR, QC], f32, tag="qrps")
                    nc.tensor.matmul(ps, lhsT=wq_sb, rhs=qc,
                                     start=True, stop=True)
                    nc.scalar.copy(out=qrT_sb[:, c * QC:(c + 1) * QC], in_=ps)

            # ---- per row-tile: scores, top-16, merge locals, sorted out ----
            # prefetch qr for the first few tiles before the remaining kT loads
            ensure_qr(NTILES_SKIP * 128, NTILES_SKIP * 128 + 2 * QC)

            # kr chunk c is needed from: c0 -> (b0, i<32), c1 -> (b0, i>=32),
            # c2 -> (b1, i<32), c3 -> (b1, i>=32). Chunk work is spread one
            # op per tile iteration so Pool's in-order stream never stalls.
            enqueue_at = {4: (1, nc.vector), 40: (2, nc.gpsimd),
                          64: (3, nc.gpsimd)}
            deadline = {24: 1, 56: 2, 80: 3}
            pending = []

            for g, (b, i) in enumerate(_tiles()):
                W = 2 * i + 2
                colbase = b * T + i * 128
                if g in enqueue_at:
                    cid, eng = enqueue_at[g]
                    pending.extend(ksum_steps(cid, eng))
                if g in deadline:
                    for s in pending:
                        s()
                    pending.clear()
                elif pending:
                    pending.pop(0)()
                ensure_qr(colbase, colbase + 128)
                ps = sc_ps.tile([128, 128], f32, tag="scps")
                nc.tensor.matmul(ps[:, :W], lhsT=qrT_sb[:, colbase:colbase + 128],
                                 rhs=krT_sb[:, b * NB:b * NB + W],
                                 start=True, stop=False)
                # rows p<64 of this tile must not see block 2i+1
                nc.tensor.matmul(ps[:, 2 * i + 1:2 * i + 2], lhsT=zap_sb,
                                 rhs=one_sb, start=False, stop=True)

                sc = scpool.tile([128, 128], f32, tag="sc")
                nc.scalar.copy(out=sc[:, :W], in_=ps[:, :W])

                v = small.tile([128, 16], f32, tag="v")
                sc2 = scpool.tile([128, 128], f32, tag="sc2")
                nc.vector.max(out=v[:, 0:8], in_=sc[:, :W])
                nc.vector.match_replace(out=sc2[:, :W], in_to_replace=v[:, 0:8],
                                        in_values=sc[:, :W], imm_value=NEG_BIG)
                nc.vector.max(out=v[:, 8:16], in_=sc2[:, :W])

                # cneg[j] = -1 if sc[j] >= tau (16th largest) else 0
                cneg = scpool.tile([128, 128], f32, tag="cneg")
                nc.gpsimd.tensor_scalar(cneg[:, :W], sc[:, :W], v[:, 15:16],
                                        -1.0, op0=mybir.AluOpType.is_ge,
                                        op1=mybir.AluOpType.mult)
                # P = prefix sum of cneg: hits -1..-16 at the sorted positions
                P = scpool.tile([128, 128], f32, tag="pscan")
                nc.vector.tensor_tensor_scan(P[:, :W], cneg[:, :W], cneg[:, :W],
                                             0.0, op0=mybir.AluOpType.add,
                                             op1=mybir.AluOpType.bypass)

                gi = g % GB
                if gi == 0:
                    ogrp = ogpool.tile([128, GB, SEL], mybir.dt.uint32, tag="ogrp")
                o2a = ogrp[:, gi:gi + 1, 0:8].rearrange("p a b -> p (a b)")
                o2b = ogrp[:, gi:gi + 1, 8:16].rearrange("p a b -> p (a b)")
                nc.vector.max_index(out=o2a, in_max=cdesc_sb[:, 0:8],
                                    in_values=P[:, :W])
                nc.vector.max_index(out=o2b, in_max=cdesc_sb[:, 8:16],
                                    in_values=P[:, :W])
                if gi == GB - 1:
                    # clamp each tile's last slot with t_blk-1, then store
                    lastcol = ogrp[:, :, 15:16].rearrange("p a b -> p (a b)")
                    nc.vector.tensor_tensor(lastcol, lastcol,
                                            loc1_sb[:, g - GB + 1:g + 1],
                                            mybir.AluOpType.min)
                    jb = b * TPB + (i - GB + 1)
                    nc.sync.dma_start(out=out_v[:, jb:jb + GB, :], in_=ogrp)
    nc.compile()
    return nc


def _shard_inputs(Q, K, Wq, Wk):
    early, loc1, cdesc, zap, one = _static_tables()
    early = early.astype(np.uint32)
    in_maps = []
    for h in range(H):
        qT = np.ascontiguousarray(
            Q[:, :, GROUPS * h, :].reshape(ROWS, D).T)
        kT = np.ascontiguousarray(K[:, :, h, :].reshape(ROWS, D).T)
        in_maps.append({
            "qT": qT.astype(np.float32),
            "kT": kT.astype(np.float32),
            "wq": np.ascontiguousarray(Wq[h]).astype(np.float32),
            "wks": np.ascontiguousarray(Wk[h] / 64.0).astype(np.float32),
            "loc1": loc1, "cdesc": cdesc, "zap": zap, "one": one,
            "early": early,
        })
    return in_maps


def kernel(Q, K, Wq, Wk, logit_scale=None, block_size=64, selected_blocks=16,
           groups=4, **_unused):
    assert int(block_size) == BS and int(selected_blocks) == SEL
    assert int(groups) == GROUPS
    Q = np.asarray(Q, np.float32)
    K = np.asarray(K, np.float32)
    Wq = np.asarray(Wq, np.float32)
    Wk = np.asarray(Wk, np.float32)
    # exp(logit_scale) > 0 scales scores per-head only -> ranking unchanged.

    if "nc" not in _CACHE:
        _CACHE["nc"] = build_program()
    nc = _CACHE["nc"]

    in_maps = _shard_inputs(Q, K, Wq, Wk)
    res = run_bass_kernel_spmd(nc, in_maps, core_ids=list(range(H)))
    outs = [res.results[h]["out"] for h in range(H)]          # [ROWS, SEL] i32
    out = np.stack(outs, axis=1).reshape(B, T, H, SEL)
    return out.astype(np.int32)


if __name__ == "__main__":
    rng = np.random.default_rng(0)
    Q = rng.standard_normal((B, T, HQ, D)).astype(np.float32)
    K = rng.standard_normal((B, T, H, D)).astype(np.float32)
    Wq = (rng.standard_normal((H, D, DR)) * 0.02).astype(np.float32)
    Wk = (rng.standard_normal((H, D, DR)) * 0.02).astype(np.float32)
    out = kernel(Q=Q, K=K, Wq=Wq, Wk=Wk)
    print("kernel ran:", out.shape, out.dtype)
